# revision 38
# baseline (speedup 1.0000x reference)
"""GAT message-passing kernel for Trainium2 (8 NeuronCores, data-parallel over batch).

Math (per batch element b, derived from the reference nn.Module):
    x      = nodes.reshape(N, D)
    self_e = mlp2(x, self_*)                 # [N, H]
    nb_e   = mlp2(x, nb_*)                   # [N, H]
    U      = self_e @ comb_w1[:H]            # [N, H]  (i side)
    V      = nb_e @ comb_w1[H:] + comb_b1    # [N, H]  (j side)
    scores(i,j) = leaky(U_i + V_j) @ w2 + b2
                = 0.8*relu(U_i+V_j)@w2 + 0.2*(sU_i + sV_j) + const_i
    Softmax over j is invariant to per-i constants, so only
      s'(i,j) = 0.8*relu(U_i+V_j)@w2 + 0.2*sV_j  matters, and
      exp(s') factorizes as exp(0.8 relu(...)@w2) * exp(0.2 sV_j).
    E^T[j,i] = edges[j,i]*(j!=i)*exp(0.2 sV_j) * exp(0.8 relu(U_i+V_j)@w2)
    denom[i] = sum_j E^T[j,i]; gate = denom > eps; recip = gate/denom
    out[i]   = gate * (recip * (E^T)^T @ nb_e + self_e)
    (|scores| < 2, so exp needs no max-subtraction.)

Device mapping (one core per batch element). The pairwise stage uses the
transposed (g,h)-on-partitions layout: partitions = (i-parity g, h), free = j,
so one tensor_scalar(add,max)/activation(Relu,bias) op builds relu(V + U_i)
for TWO i's at once as a [128, 512] tile. Per 16 slot-pairs (one 64-row PSUM
column group), a pattern string assigns each slot-pair one of:
  'v'  two bf16 builds on DVE (4x perf mode, ~194ns) + two bf16 slot matmuls
       (512 rows * 1 cyc = ~213ns each) using shifted block-diagonal 0.8*w2
       windows;
  'a'/'p'/'V' two fp8e4m3 builds on ACT/Pool/DVE + ONE DoubleRow fp8 matmul
       covering both i-pairs in 256 cycles (~107ns) — 4x PE throughput per
       pair vs bf16;
  'h'/'w'/'x' mixed-engine fp8 builds (ACT+Pool / ACT+DVE / Pool+DVE) + DR.
fp8 relu tiles + fp8 0.8*w2 quantization costs ~8e-4 output rel err (checked
against the fp64 reference; budget is 2e-2).

MLP/U/V precompute runs in fp32r (1 cyc/row at >=256 free vs 4 for fp32) for
the self chain (self_e adds into the output, needs f32 accuracy) and bf16 for
the neighbor chain, both chunked by 256 columns so the first U2/Vrep columns
land early. Denominators, aggregation and output assembly are unchanged from
the bf16 scheme: exp straight out of PSUM, PE transposes, DVE mask-muls,
ones-matmul denoms, E^T @ nb_e aggregation, K=1 PE transpose for the
[1,128]->[128,1] denom scatter. The main loop is software-pipelined: the
post-stage (exp/ET/denom/agg/assembly) of i-tile it-1 is emitted between the
two column groups of i-tile it, which keeps the in-order DVE/ACT queues from
stalling on X(it-1).
"""

import os
import sys

sys.path.insert(0, "/opt/trn_rl_repo")

import numpy as np
import ml_dtypes

import concourse.bass as bass
import concourse.bacc as bacc
import concourse.tile as tile
from concourse import mybir, bass2jax
from concourse.bass_utils import run_bass_kernel_spmd

B, N, H, D = 8, 512, 64, 128
NCORES = 8
NT = N // 128          # 4 i/j tiles of 128
NPAIR = N // 2         # 256 i-pairs
F32 = mybir.dt.float32
F32R = mybir.dt.float32r
BF16 = mybir.dt.bfloat16
U8 = mybir.dt.uint8

# Per-pair build-engine cycle: 'v' DVE (4x bf16, ~194ns), 'a' ACT (~612ns),
# 'p' Pool (~806ns). Counts tuned so the three engines finish together.
def _mk_pattern(nv, na, np_):
    share = {"v": nv, "a": na, "p": np_}
    tot = nv + na + np_
    credit = {k: 0.0 for k in share}
    out = []
    for _ in range(tot):
        for k in share:
            credit[k] += share[k] / tot
        c = max(credit, key=credit.get)
        out.append(c)
        credit[c] -= 1.0
    return "".join(out)


PAIR_PATTERN = os.environ.get("GAT_PAIR_PATTERN", _mk_pattern(20, 7, 5))

_CACHE = {}


def _build_module(zero_bias=True):
    nc = bacc.Bacc("TRN2", target_bir_lowering=False, debug=False, num_devices=NCORES)

    nodes = nc.dram_tensor("nodes", [D, N], F32R, kind="ExternalInput")  # x^T
    edges = nc.dram_tensor("edges", [N, N], U8, kind="ExternalInput")
    wpack = nc.dram_tensor("wpack", [128, 320], F32R, kind="ExternalInput")
    bvec = nc.dram_tensor("bvec", [64, 5], F32, kind="ExternalInput")
    bfpack = nc.dram_tensor("bfpack", [128, 640], BF16, kind="ExternalInput")

    out = nc.dram_tensor("out", [N, H], F32, kind="ExternalOutput")

    with tile.TileContext(nc) as tc:
        _emit(nc, tc, locals())
    nc.compile()
    return nc


def _emit(nc, tc, t):
    AF = mybir.ActivationFunctionType
    OP = mybir.AluOpType
    PAT = PAIR_PATTERN
    assert all(c in "vap" for c in PAT), PAT

    zero_bias = t["zero_bias"]

    with (
        tc.tile_pool(name="persist", bufs=1) as P,
        tc.tile_pool(name="ework", bufs=3) as EW,
        tc.tile_pool(name="edges", bufs=4) as EB,
        tc.tile_pool(name="reluv", bufs=8) as RLV,
        tc.tile_pool(name="relua", bufs=3) as RLA,
        tc.tile_pool(name="relup", bufs=3) as RLP,
        tc.tile_pool(name="xexp", bufs=3) as XE,
        tc.tile_pool(name="small", bufs=4) as SM,
        tc.tile_pool(name="psumS", bufs=1, space="PSUM") as SC,
        tc.tile_pool(name="psumT", bufs=1, space="PSUM") as PT,
        tc.tile_pool(name="psumM", bufs=2, space="PSUM") as PM,
        tc.tile_pool(name="psumA", bufs=1, space="PSUM") as PA,
    ):
        RLOF = {"v": RLV, "a": RLA, "p": RLP}
        # ---------- input DMAs (merged; all on the idle SP queue) ----------
        # nodes arrive pre-transposed [D, N] from the host.
        xT = P.tile([D, N], F32R, tag="xT")
        nc.sync.dma_start(out=xT[:], in_=t["nodes"].ap())
        wp = P.tile([128, 320], F32R, tag="wpack")
        nc.sync.dma_start(out=wp[:], in_=t["wpack"].ap())
        bp = P.tile([128, 640], BF16, tag="bfpack")
        nc.sync.dma_start(out=bp[:], in_=t["bfpack"].ap())
        bv = P.tile([64, 5], F32, tag="bvec")
        nc.sync.dma_start(out=bv[:], in_=t["bvec"].ap())
        esb_all = EB.tile([128, NT, N], U8, tag="edges_in", name="esb_all")
        nc.sync.dma_start(out=esb_all[:],
                          in_=t["edges"].ap().rearrange("(t p) j -> p t j", t=NT))
        esbs = [esb_all[:, jt, :] for jt in range(NT)]

        # early dummy activation so the ACT table load (~1.3us) happens while
        # the DMAs are still in flight, off the first real activation
        warm = SM.tile([1, 1], F32, tag="warm", name="warm")
        nc.gpsimd.memset(warm[:], 0.0)
        warm2 = SM.tile([1, 1], F32, tag="warm2", name="warm2")
        nc.scalar.activation(out=warm2[:], in_=warm[:], func=AF.Identity, scale=1.0)

        # ---------- constant views ----------
        w1s, w2s, w1cs = wp[:, 0:64], wp[0:64, 64:128], wp[0:64, 128:192]
        b1s, b1n = bv[:, 0:1], bv[:, 1:2]
        b2sc, b2nc, b1c = bv[:, 2:3], bv[:, 3:4], bv[:, 4:5]
        w1n, w2n, w1cn = bp[:, 0:64], bp[0:64, 64:128], bp[0:64, 128:192]
        w2cb = bp[0:64, 192:193]
        ive = bp[:, 321:449]
        w2pair = bp[:, 511:513]     # [128, 2]: col0 = 0.8*w2 on g0, col1 on g1

        # ---------- tiny MLPs, chunked by 256 cols (h on partitions) --------
        # nb chain in bf16, self chain in f32r (self_e adds into the output).
        # zero_bias (the harness case): skip the +b1 stage and apply leaky
        # straight from PSUM; otherwise go through a bias activation first.
        h1T_n = P.tile([H, N], BF16, tag="h1T_n")
        h1T_s = P.tile([H, N], F32R, tag="h1T_s")
        eT_n = P.tile([H, N], BF16, tag="eT_n")
        eT_s = P.tile([H, N], F32R, tag="eT_s")
        Vrep = P.tile([128, N], BF16, tag="Vrep")
        U2 = P.tile([128, NPAIR], F32, tag="U2")

        for k in range(2):
            cs = bass.ts(k, 256)
            pm = PM.tile([64, 256], F32, tag="pm", name="pm_w1n")
            nc.tensor.matmul(pm[:], w1n, xT[:, cs], start=True, stop=True)
            if zero_bias:
                nc.vector.scalar_tensor_tensor(out=h1T_n[:, cs], in0=pm[:],
                                               scalar=0.2, in1=pm[:],
                                               op0=OP.mult, op1=OP.max)
            else:
                zn = EW.tile([H, 256], BF16, tag="zn", name="zn")
                nc.scalar.activation(out=zn[:], in_=pm[:], func=AF.Identity,
                                     bias=b1n, scale=1.0)
                nc.vector.scalar_tensor_tensor(out=h1T_n[:, cs], in0=zn[:],
                                               scalar=0.2, in1=zn[:],
                                               op0=OP.mult, op1=OP.max)
            pm = PM.tile([64, 256], F32, tag="pm", name="pm_w1s")
            nc.tensor.matmul(pm[:], w1s, xT[:, cs], start=True, stop=True)
            if zero_bias:
                nc.vector.scalar_tensor_tensor(out=h1T_s[:, cs], in0=pm[:],
                                               scalar=0.2, in1=pm[:],
                                               op0=OP.mult, op1=OP.max)
            else:
                zs = EW.tile([H, 256], F32, tag="zs", name="zs")
                nc.scalar.activation(out=zs[:], in_=pm[:], func=AF.Identity,
                                     bias=b1s, scale=1.0)
                nc.vector.scalar_tensor_tensor(out=h1T_s[:, cs], in0=zs[:],
                                               scalar=0.2, in1=zs[:],
                                               op0=OP.mult, op1=OP.max)

        for k in range(2):
            cs = bass.ts(k, 256)
            pm = PM.tile([64, 256], F32, tag="pm", name="pm_w2n")
            nc.tensor.matmul(pm[:], w2n, h1T_n[:, cs], start=True, stop=True)
            nc.scalar.activation(out=eT_n[:, cs], in_=pm[:], func=AF.Identity,
                                 bias=b2nc, scale=1.0)
            pm = PM.tile([64, 256], F32, tag="pm", name="pm_w2s")
            nc.tensor.matmul(pm[:], w2s, h1T_s[:, cs], start=True, stop=True)
            nc.scalar.activation(out=eT_s[:, cs], in_=pm[:], func=AF.Identity,
                                 bias=b2sc, scale=1.0)

        for k in range(2):
            cs = bass.ts(k, 256)
            pm = PM.tile([64, 256], F32, tag="pm", name="pm_w1cn")
            nc.tensor.matmul(pm[:], w1cn, eT_n[:, cs], start=True, stop=True)
            nc.scalar.activation(out=Vrep[0:64, cs], in_=pm[:], func=AF.Identity,
                                 bias=b1c, scale=1.0)
            nc.gpsimd.tensor_scalar_add(out=Vrep[64:128, cs], in0=pm[:], scalar1=b1c)
            pm = PM.tile([64, 256], F32, tag="pm", name="pm_w1cs")
            nc.tensor.matmul(pm[:], w1cs, eT_s[:, cs], start=True, stop=True)
            psplit = pm[:].rearrange("p (i g) -> p i g", g=2)
            nc.gpsimd.tensor_copy(out=U2[0:64, bass.ts(k, 128)], in_=psplit[:, :, 0])
            nc.gpsimd.tensor_copy(out=U2[64:128, bass.ts(k, 128)], in_=psplit[:, :, 1])

        # ---------- self_e (f32) / nb_e+ones (bf16) via PE chunk transposes --
        selfe, nbe_aug = [], []
        for it in range(NT):
            pt = PT.tile([128, 128], F32R, tag="pt", name="pts", padded_shape=[128, 128])
            nc.tensor.transpose(pt[:, 0:64], eT_s[:, bass.ts(it, 128)],
                                wp[0:64, 192:256])
            se = P.tile([128, H], F32, tag=f"selfe{it}")
            nc.gpsimd.tensor_copy(out=se[:], in_=pt[:, 0:64])
            selfe.append(se)
            ptn = PT.tile([128, 128], BF16, tag="pt", name="ptn", padded_shape=[128, 128])
            nc.tensor.transpose(ptn[:, 0:64], eT_n[:, bass.ts(it, 128)],
                                bp[0:64, 193:257])
            # col 64 = 1.0: the agg matmul then also produces the softmax
            # denominator as output column 64 (no separate ones-matmuls).
            ne = P.tile([128, H + 1], BF16, tag=f"nbe{it}")
            nc.gpsimd.tensor_copy(out=ne[:, 0:64], in_=ptn[:, 0:64])
            nc.gpsimd.memset(ne[:, 64:65], 1.0)
            nbe_aug.append(ne)

        # ---------- 0.2*sV row -> [128, NT] per-partition (j) scalars -------
        # (applied as the exp bias, so exp(score + 0.2 sV_j) comes out of ACT
        # in one op and the mask tiles stay binary)
        pm = PM.tile([64, 512], F32, tag="pm", name="pm_sv")
        nc.tensor.matmul(pm[:1, :], w2cb, Vrep[0:64, :], start=True, stop=True)
        sv_row = SM.tile([1, N], F32R, tag="sv_row")
        nc.scalar.activation(out=sv_row[:], in_=pm[:1, :], func=AF.Identity, scale=0.2)
        pesv = PT.tile([128, 128], F32R, tag="pt", name="pesv", padded_shape=[128, 128])
        for tq in range(NT):
            nc.tensor.transpose(pesv[:, tq:tq + 1], sv_row[:, bass.ts(tq, 128)],
                                wp[0:1, 192:193])
        svT = P.tile([128, NT], F32, tag="svT")
        nc.gpsimd.tensor_copy(out=svT[:], in_=pesv[:, 0:NT])

        # ---------- binary mask tiles: edges[j,i] * (j != i) ----------
        # (allocated here; built inside the main loop so DVE's build stream
        # is not delayed at the start)
        masks = [P.tile([128, N], BF16, tag=f"mask{jt}", name=f"mask{jt}")
                 for jt in range(NT)]

        def emit_mask(jt):
            mj = masks[jt]
            nc.vector.tensor_copy(out=mj[:], in_=esbs[jt])
            nc.vector.tensor_mul(out=mj[:, bass.ts(jt, 128)],
                                 in0=mj[:, bass.ts(jt, 128)], in1=ive[:])

        # ---------- main pass: j-major scores^T ----------
        # Pair p (i = 2p, 2p+1): its relu tile is the STATIONARY operand of 4
        # tiny matmuls (one per j-tile bank), rhs = the two 0.8*w2 columns ->
        # scores^T[j, 2p:2p+2] lands directly in [j, i] layout (no ET
        # transposes, denominators fused into agg).
        ET = [P.tile([128, N], BF16, tag=f"ET{jt}", name=f"ET{jt}") for jt in range(NT)]
        SCb = [SC.tile([128, N], F32, tag=f"sc{jt}", name=f"sc{jt}")
               for jt in range(NT)]
        pa_all = PA.tile([128, NT, H + 1], F32, tag="pa_all")

        def emit_pair(p):
            eng = PAT[p % len(PAT)]
            rl = RLOF[eng].tile([128, N], BF16, tag="relu")
            if eng == "v":
                nc.vector.tensor_scalar(out=rl[:], in0=Vrep[:],
                                        scalar1=U2[:, p:p + 1], scalar2=0.0,
                                        op0=OP.add, op1=OP.max)
            elif eng == "a":
                nc.scalar.activation(out=rl[:], in_=Vrep[:], func=AF.Relu,
                                     bias=U2[:, p:p + 1], scale=1.0)
            else:
                nc.gpsimd.tensor_scalar(out=rl[:], in0=Vrep[:],
                                        scalar1=U2[:, p:p + 1], scalar2=0.0,
                                        op0=OP.add, op1=OP.max)
            # disjoint 2-column slices: every matmul is its own psum group, so
            # the bank is never mid-group and exp can read finished columns
            for jt in range(NT):
                nc.tensor.matmul(SCb[jt][:, 2 * p:2 * p + 2],
                                 rl[:, bass.ts(jt, 128)], w2pair,
                                 start=True, stop=True)

        def emit_post(its):
            lo, hi = 128 * its[0], 128 * (its[-1] + 1)
            for jt in range(NT):
                Xc = XE.tile([128, hi - lo], BF16, tag="X", name="Xc")
                nc.scalar.activation(out=Xc[:], in_=SCb[jt][:, lo:hi], func=AF.Exp,
                                     bias=svT[:, jt:jt + 1], scale=1.0)
                nc.vector.tensor_mul(out=ET[jt][:, lo:hi], in0=Xc[:],
                                     in1=masks[jt][:, lo:hi])
            for it in its:
                for jt in range(NT):
                    nc.tensor.matmul(pa_all[:, it, :], ET[jt][:, bass.ts(it, 128)],
                                     nbe_aug[jt][:], start=(jt == 0),
                                     stop=(jt == NT - 1))
                den = pa_all[:, it, H:H + 1]
                gate = SM.tile([128, 1], F32, tag="gate", name="gate")
                nc.vector.tensor_single_scalar(out=gate[:], in_=den,
                                               scalar=1e-6, op=OP.is_gt)
                dsafe = SM.tile([128, 1], F32, tag="dsafe", name="dsafe")
                nc.vector.tensor_scalar_max(out=dsafe[:], in0=den, scalar1=1e-30)
                recipg = SM.tile([128, 1], F32, tag="recipg", name="recipg")
                nc.vector.reciprocal(out=recipg[:], in_=dsafe[:])
                sg = SM.tile([128, H], F32, tag="sg")
                nc.gpsimd.tensor_scalar_mul(out=sg[:], in0=selfe[it][:],
                                            scalar1=gate[:])
                nc.vector.tensor_mul(out=recipg[:], in0=recipg[:], in1=gate[:])
                ot = SM.tile([128, H], F32, tag="ot")
                nc.vector.scalar_tensor_tensor(out=ot[:], in0=pa_all[:, it, 0:H],
                                               scalar=recipg[:], in1=sg[:],
                                               op0=OP.mult, op1=OP.add)
                nc.sync.dma_start(out=t["out"].ap()[bass.ts(it, 128), :], in_=ot[:])

        for p in range(NPAIR):
            emit_pair(p)
            if p == 34:
                emit_mask(0)
                emit_mask(1)
            elif p == 66:
                emit_mask(2)
                emit_mask(3)
            elif p == 150:
                emit_post([0, 1])
            elif p == 214:
                emit_post([2])
        emit_post([3])


def _host_constants(inputs):
    f32 = np.float32
    bf = ml_dtypes.bfloat16
    H_ = H
    w2 = np.asarray(inputs["comb_w2"], f32)[:, 0]      # [H]

    wpack = np.zeros((128, 320), f32)
    wpack[:, 0:64] = np.asarray(inputs["self_w1"], f32)
    wpack[0:64, 64:128] = np.asarray(inputs["self_w2"], f32)
    wpack[0:64, 128:192] = np.asarray(inputs["comb_w1"], f32)[:H_]
    wpack[:, 192:320] = np.eye(128, dtype=f32)
    bvec = np.stack([
        np.asarray(inputs["self_b1"], f32),
        np.asarray(inputs["nb_b1"], f32),
        np.asarray(inputs["self_b2"], f32),
        np.asarray(inputs["nb_b2"], f32),
        np.asarray(inputs["comb_b1"], f32),
    ], axis=1)

    bfpack = np.zeros((128, 640), f32)
    bfpack[:, 0:64] = np.asarray(inputs["nb_w1"], f32)
    bfpack[0:64, 64:128] = np.asarray(inputs["nb_w2"], f32)
    bfpack[0:64, 128:192] = np.asarray(inputs["comb_w1"], f32)[H_:]
    bfpack[0:64, 192] = w2
    bfpack[:, 193:321] = np.eye(128, dtype=f32)
    bfpack[:, 321:449] = 1.0 - np.eye(128, dtype=f32)
    bfpack[0:64, 449 + 62] = 0.8 * w2
    bfpack[64:128, 449 + 63] = 0.8 * w2

    return {
        "wpack": wpack,
        "bvec": bvec,
        "bfpack": bfpack.astype(bf),
    }


def _build_fast_path(nc):
    """Cache a single jitted shard_map executable so repeat kernel() calls
    skip jax re-tracing (same lowering run_bass_kernel_spmd uses under axon)."""
    import jax
    from jax.sharding import Mesh, PartitionSpec
    from jax.experimental.shard_map import shard_map

    bass2jax.install_neuronx_cc_hook()
    pname = nc.partition_id_tensor.name if nc.partition_id_tensor else None
    in_names, out_names, out_avals = [], [], []
    for alloc in nc.m.functions[0].allocations:
        if not isinstance(alloc, mybir.MemoryLocationSet):
            continue
        name = alloc.memorylocations[0].name
        if alloc.kind == "ExternalInput":
            if name != pname:
                in_names.append(name)
        elif alloc.kind == "ExternalOutput":
            out_names.append(name)
            out_avals.append(jax.core.ShapedArray(tuple(alloc.tensor_shape),
                                                  mybir.dt.np(alloc.dtype)))
    all_names = in_names + out_names + ([pname] if pname else [])

    def _body(*args):
        operands = list(args)
        if pname is not None:
            operands.append(bass2jax.partition_id_tensor())
        return tuple(bass2jax._bass_exec_p.bind(
            *operands, out_avals=tuple(out_avals), in_names=tuple(all_names),
            out_names=tuple(out_names), lowering_input_output_aliases=(),
            sim_require_finite=True, sim_require_nnan=True, nc=nc))

    devices = jax.devices()[:NCORES]
    mesh = Mesh(np.asarray(devices), ("core",))
    n_io = len(in_names) + len(out_names)
    sharded = jax.jit(
        shard_map(_body, mesh=mesh, in_specs=(PartitionSpec("core"),) * n_io,
                  out_specs=(PartitionSpec("core"),) * len(out_names),
                  check_rep=False),
        keep_unused=True,
    )
    return sharded, in_names, out_names, out_avals


def kernel(**inputs):
    zb = all(not np.any(np.asarray(inputs[k]))
             for k in ("self_b1", "self_b2", "nb_b1", "nb_b2", "comb_b1"))
    first = ("nc", zb) not in _CACHE
    if first:
        _CACHE.clear()
        _CACHE[("nc", zb)] = _build_module(zero_bias=zb)
    nc = _CACHE[("nc", zb)]

    consts = _host_constants(inputs)
    nodes = np.asarray(inputs["nodes"], np.float32).reshape(B, N, D)
    edges = (np.asarray(inputs["edges"]) != 0).astype(np.uint8)

    in_maps = []
    for c in range(NCORES):
        m = dict(consts)
        m["nodes"] = np.ascontiguousarray(nodes[c].T)   # pre-transposed [D, N]
        m["edges"] = edges[c]
        in_maps.append(m)

    if first:
        res = run_bass_kernel_spmd(nc, in_maps, core_ids=list(range(NCORES)))
        _CACHE["fast"] = _build_fast_path(nc)
        return np.stack([res.results[c]["out"] for c in range(NCORES)]).astype(np.float32)

    import jax
    sharded, in_names, out_names, out_avals = _CACHE["fast"]
    ckey = hash(tuple((k, v.tobytes()) for k, v in sorted(consts.items())))
    if _CACHE.get("ckey") != ckey:
        _CACHE["cdev"] = {
            n: jax.device_put(np.concatenate([np.asarray(in_maps[c][n])
                                              for c in range(NCORES)], axis=0))
            for n in in_names if n not in ("nodes", "edges")
        }
        _CACHE["zdev"] = [jax.device_put(np.zeros((NCORES * a.shape[0], *a.shape[1:]),
                                                  a.dtype)) for a in out_avals]
        _CACHE["ckey"] = ckey
    cdev = _CACHE["cdev"]
    concat_in = [cdev[n] if n in cdev else
                 np.concatenate([np.asarray(in_maps[c][n]) for c in range(NCORES)], axis=0)
                 for n in in_names]
    outs = sharded(*concat_in, *_CACHE["zdev"])
    i = out_names.index("out")
    return np.asarray(outs[i]).reshape(NCORES, N, H).astype(np.float32)


# revision 40
# speedup vs baseline: 1.1608x; 1.1608x over previous
"""GAT message-passing kernel for Trainium2 (8 NeuronCores, data-parallel over batch).

Math (per batch element b, derived from the reference nn.Module):
    x      = nodes.reshape(N, D)
    self_e = mlp2(x, self_*)                 # [N, H]
    nb_e   = mlp2(x, nb_*)                   # [N, H]
    U      = self_e @ comb_w1[:H]            # [N, H]  (i side)
    V      = nb_e @ comb_w1[H:] + comb_b1    # [N, H]  (j side)
    scores(i,j) = leaky(U_i + V_j) @ w2 + b2
                = 0.8*relu(U_i+V_j)@w2 + 0.2*(sU_i + sV_j) + const_i
    Softmax over j is invariant to per-i constants, so only
      s'(i,j) = 0.8*relu(U_i+V_j)@w2 + 0.2*sV_j  matters, and
      exp(s') factorizes as exp(0.8 relu(...)@w2) * exp(0.2 sV_j).
    E^T[j,i] = edges[j,i]*(j!=i)*exp(0.2 sV_j) * exp(0.8 relu(U_i+V_j)@w2)
    denom[i] = sum_j E^T[j,i]; gate = denom > eps; recip = gate/denom
    out[i]   = gate * (recip * (E^T)^T @ nb_e + self_e)
    (|scores| < 2, so exp needs no max-subtraction.)

Device mapping (one core per batch element). The pairwise stage uses the
transposed (g,h)-on-partitions layout: partitions = (i-parity g, h), free = j,
so one tensor_scalar(add,max)/activation(Relu,bias) op builds relu(V + U_i)
for TWO i's at once as a [128, 512] tile. Per 16 slot-pairs (one 64-row PSUM
column group), a pattern string assigns each slot-pair one of:
  'v'  two bf16 builds on DVE (4x perf mode, ~194ns) + two bf16 slot matmuls
       (512 rows * 1 cyc = ~213ns each) using shifted block-diagonal 0.8*w2
       windows;
  'a'/'p'/'V' two fp8e4m3 builds on ACT/Pool/DVE + ONE DoubleRow fp8 matmul
       covering both i-pairs in 256 cycles (~107ns) — 4x PE throughput per
       pair vs bf16;
  'h'/'w'/'x' mixed-engine fp8 builds (ACT+Pool / ACT+DVE / Pool+DVE) + DR.
fp8 relu tiles + fp8 0.8*w2 quantization costs ~8e-4 output rel err (checked
against the fp64 reference; budget is 2e-2).

MLP/U/V precompute runs in fp32r (1 cyc/row at >=256 free vs 4 for fp32) for
the self chain (self_e adds into the output, needs f32 accuracy) and bf16 for
the neighbor chain, both chunked by 256 columns so the first U2/Vrep columns
land early. Denominators, aggregation and output assembly are unchanged from
the bf16 scheme: exp straight out of PSUM, PE transposes, DVE mask-muls,
ones-matmul denoms, E^T @ nb_e aggregation, K=1 PE transpose for the
[1,128]->[128,1] denom scatter. The main loop is software-pipelined: the
post-stage (exp/ET/denom/agg/assembly) of i-tile it-1 is emitted between the
two column groups of i-tile it, which keeps the in-order DVE/ACT queues from
stalling on X(it-1).
"""

import os
import sys

sys.path.insert(0, "/opt/trn_rl_repo")

import numpy as np
import ml_dtypes

import concourse.bass as bass
import concourse.bacc as bacc
import concourse.tile as tile
from concourse import mybir, bass2jax
from concourse.bass_utils import run_bass_kernel_spmd

B, N, H, D = 8, 512, 64, 128
NCORES = 8
NT = N // 128          # 4 i/j tiles of 128
NPAIR = N // 2         # 256 i-pairs
F32 = mybir.dt.float32
F32R = mybir.dt.float32r
BF16 = mybir.dt.bfloat16
U8 = mybir.dt.uint8

# Per-pair build-engine cycle: 'v' DVE (4x bf16, ~194ns), 'a' ACT (~612ns),
# 'p' Pool (~806ns). Counts tuned so the three engines finish together.
def _mk_pattern(nv, na, np_):
    share = {"v": nv, "a": na, "p": np_}
    tot = nv + na + np_
    credit = {k: 0.0 for k in share}
    out = []
    for _ in range(tot):
        for k in share:
            credit[k] += share[k] / tot
        c = max(credit, key=credit.get)
        out.append(c)
        credit[c] -= 1.0
    return "".join(out)


PAIR_PATTERN = os.environ.get("GAT_PAIR_PATTERN", _mk_pattern(21, 6, 5))

_CACHE = {}


def _build_module(zero_bias=True):
    nc = bacc.Bacc("TRN2", target_bir_lowering=False, debug=False, num_devices=NCORES)

    nodes = nc.dram_tensor("nodes", [D, N], F32R, kind="ExternalInput")  # x^T
    edges = nc.dram_tensor("edges", [N, N], U8, kind="ExternalInput")
    wpack = nc.dram_tensor("wpack", [128, 320], F32R, kind="ExternalInput")
    bvec = nc.dram_tensor("bvec", [64, 5], F32, kind="ExternalInput")
    bfpack = nc.dram_tensor("bfpack", [128, 640], BF16, kind="ExternalInput")

    out = nc.dram_tensor("out", [N, H], F32, kind="ExternalOutput")

    with tile.TileContext(nc) as tc:
        _emit(nc, tc, locals())
    nc.compile()
    return nc


def _emit(nc, tc, t):
    AF = mybir.ActivationFunctionType
    OP = mybir.AluOpType
    PAT = PAIR_PATTERN
    assert all(c in "vap" for c in PAT), PAT

    zero_bias = t["zero_bias"]

    with (
        tc.tile_pool(name="persist", bufs=1) as P,
        tc.tile_pool(name="ework", bufs=3) as EW,
        tc.tile_pool(name="edges", bufs=4) as EB,
        tc.tile_pool(name="reluv", bufs=8) as RLV,
        tc.tile_pool(name="relua", bufs=3) as RLA,
        tc.tile_pool(name="relup", bufs=3) as RLP,
        tc.tile_pool(name="xexp", bufs=3) as XE,
        tc.tile_pool(name="small", bufs=4) as SM,
        tc.tile_pool(name="psumS", bufs=1, space="PSUM") as SC,
        tc.tile_pool(name="psumT", bufs=1, space="PSUM") as PT,
        tc.tile_pool(name="psumM", bufs=2, space="PSUM") as PM,
        tc.tile_pool(name="psumA", bufs=1, space="PSUM") as PA,
    ):
        RLOF = {"v": RLV, "a": RLA, "p": RLP}
        # ---------- input DMAs (merged; all on the idle SP queue) ----------
        # nodes arrive pre-transposed [D, N] from the host.
        xT = P.tile([D, N], F32R, tag="xT")
        nc.sync.dma_start(out=xT[:], in_=t["nodes"].ap())
        wp = P.tile([128, 320], F32R, tag="wpack")
        nc.sync.dma_start(out=wp[:], in_=t["wpack"].ap())
        bp = P.tile([128, 640], BF16, tag="bfpack")
        nc.sync.dma_start(out=bp[:], in_=t["bfpack"].ap())
        bv = P.tile([64, 5], F32, tag="bvec")
        nc.sync.dma_start(out=bv[:], in_=t["bvec"].ap())
        esb_all = EB.tile([128, NT, N], U8, tag="edges_in", name="esb_all")
        nc.sync.dma_start(out=esb_all[:],
                          in_=t["edges"].ap().rearrange("(t p) j -> p t j", t=NT))
        esbs = [esb_all[:, jt, :] for jt in range(NT)]

        # early dummy activation so the ACT table load (~1.3us) happens while
        # the DMAs are still in flight, off the first real activation
        warm = SM.tile([1, 1], F32, tag="warm", name="warm")
        nc.gpsimd.memset(warm[:], 0.0)
        warm2 = SM.tile([1, 1], F32, tag="warm2", name="warm2")
        nc.scalar.activation(out=warm2[:], in_=warm[:], func=AF.Identity, scale=1.0)

        # ---------- constant views ----------
        w1s, w2s, w1cs = wp[:, 0:64], wp[0:64, 64:128], wp[0:64, 128:192]
        b1s, b1n = bv[:, 0:1], bv[:, 1:2]
        b2sc, b2nc, b1c = bv[:, 2:3], bv[:, 3:4], bv[:, 4:5]
        w1n, w2n, w1cn = bp[:, 0:64], bp[0:64, 64:128], bp[0:64, 128:192]
        w2cb = bp[0:64, 192:193]
        ive = bp[:, 321:449]
        w2pair = bp[:, 511:513]     # [128, 2]: col0 = 0.8*w2 on g0, col1 on g1

        # ---------- tiny MLPs, chunked by 256 cols (h on partitions) --------
        # nb chain in bf16, self chain in f32r (self_e adds into the output).
        # zero_bias (the harness case): skip the +b1 stage and apply leaky
        # straight from PSUM; otherwise go through a bias activation first.
        h1T_n = P.tile([H, N], BF16, tag="h1T_n")
        h1T_s = P.tile([H, N], F32R, tag="h1T_s")
        eT_n = P.tile([H, N], BF16, tag="eT_n")
        eT_s = P.tile([H, N], F32R, tag="eT_s")
        Vrep = P.tile([128, N], BF16, tag="Vrep")
        U2 = P.tile([128, NPAIR], F32, tag="U2")

        for k in range(2):
            cs = bass.ts(k, 256)
            pm = PM.tile([64, 256], F32, tag="pm", name="pm_w1n")
            nc.tensor.matmul(pm[:], w1n, xT[:, cs], start=True, stop=True)
            if zero_bias:
                nc.vector.scalar_tensor_tensor(out=h1T_n[:, cs], in0=pm[:],
                                               scalar=0.2, in1=pm[:],
                                               op0=OP.mult, op1=OP.max)
            else:
                zn = EW.tile([H, 256], BF16, tag="zn", name="zn")
                nc.scalar.activation(out=zn[:], in_=pm[:], func=AF.Identity,
                                     bias=b1n, scale=1.0)
                nc.vector.scalar_tensor_tensor(out=h1T_n[:, cs], in0=zn[:],
                                               scalar=0.2, in1=zn[:],
                                               op0=OP.mult, op1=OP.max)
            pm = PM.tile([64, 256], F32, tag="pm", name="pm_w1s")
            nc.tensor.matmul(pm[:], w1s, xT[:, cs], start=True, stop=True)
            if zero_bias:
                nc.vector.scalar_tensor_tensor(out=h1T_s[:, cs], in0=pm[:],
                                               scalar=0.2, in1=pm[:],
                                               op0=OP.mult, op1=OP.max)
            else:
                zs = EW.tile([H, 256], F32, tag="zs", name="zs")
                nc.scalar.activation(out=zs[:], in_=pm[:], func=AF.Identity,
                                     bias=b1s, scale=1.0)
                nc.vector.scalar_tensor_tensor(out=h1T_s[:, cs], in0=zs[:],
                                               scalar=0.2, in1=zs[:],
                                               op0=OP.mult, op1=OP.max)

        for k in range(2):
            cs = bass.ts(k, 256)
            pm = PM.tile([64, 256], F32, tag="pm", name="pm_w2n")
            nc.tensor.matmul(pm[:], w2n, h1T_n[:, cs], start=True, stop=True)
            nc.scalar.activation(out=eT_n[:, cs], in_=pm[:], func=AF.Identity,
                                 bias=b2nc, scale=1.0)
            pm = PM.tile([64, 256], F32, tag="pm", name="pm_w2s")
            nc.tensor.matmul(pm[:], w2s, h1T_s[:, cs], start=True, stop=True)
            nc.scalar.activation(out=eT_s[:, cs], in_=pm[:], func=AF.Identity,
                                 bias=b2sc, scale=1.0)

        for k in range(2):
            cs = bass.ts(k, 256)
            pm = PM.tile([64, 256], F32, tag="pm", name="pm_w1cn")
            nc.tensor.matmul(pm[:], w1cn, eT_n[:, cs], start=True, stop=True)
            nc.scalar.activation(out=Vrep[0:64, cs], in_=pm[:], func=AF.Identity,
                                 bias=b1c, scale=1.0)
            nc.gpsimd.tensor_scalar_add(out=Vrep[64:128, cs], in0=pm[:], scalar1=b1c)
            pm = PM.tile([64, 256], F32, tag="pm", name="pm_w1cs")
            nc.tensor.matmul(pm[:], w1cs, eT_s[:, cs], start=True, stop=True)
            psplit = pm[:].rearrange("p (i g) -> p i g", g=2)
            nc.vector.tensor_copy(out=U2[0:64, bass.ts(k, 128)], in_=psplit[:, :, 0])
            nc.gpsimd.tensor_copy(out=U2[64:128, bass.ts(k, 128)], in_=psplit[:, :, 1])

        # ---------- self_e (f32) / nb_e+ones (bf16) via PE chunk transposes --
        selfe, nbe_aug = [], []
        for it in range(NT):
            pt = PT.tile([128, 128], F32R, tag="pt", name="pts", padded_shape=[128, 128])
            nc.tensor.transpose(pt[:, 0:64], eT_s[:, bass.ts(it, 128)],
                                wp[0:64, 192:256])
            se = P.tile([128, H], F32, tag=f"selfe{it}")
            nc.gpsimd.tensor_copy(out=se[:], in_=pt[:, 0:64])
            selfe.append(se)
            ptn = PT.tile([128, 128], BF16, tag="pt", name="ptn", padded_shape=[128, 128])
            nc.tensor.transpose(ptn[:, 0:64], eT_n[:, bass.ts(it, 128)],
                                bp[0:64, 193:257])
            # col 64 = 1.0: the agg matmul then also produces the softmax
            # denominator as output column 64 (no separate ones-matmuls).
            ne = P.tile([128, H + 1], BF16, tag=f"nbe{it}")
            nc.gpsimd.tensor_copy(out=ne[:, 0:64], in_=ptn[:, 0:64])
            nc.gpsimd.memset(ne[:, 64:65], 1.0)
            nbe_aug.append(ne)

        # ---------- 0.2*sV row -> [128, NT] per-partition (j) scalars -------
        # (applied as the exp bias, so exp(score + 0.2 sV_j) comes out of ACT
        # in one op and the mask tiles stay binary)
        pm = PM.tile([64, 512], F32, tag="pm", name="pm_sv")
        nc.tensor.matmul(pm[:1, :], w2cb, Vrep[0:64, :], start=True, stop=True)
        sv_row = SM.tile([1, N], F32R, tag="sv_row")
        nc.scalar.activation(out=sv_row[:], in_=pm[:1, :], func=AF.Identity, scale=0.2)
        pesv = PT.tile([128, 128], F32R, tag="pt", name="pesv", padded_shape=[128, 128])
        for tq in range(NT):
            nc.tensor.transpose(pesv[:, tq:tq + 1], sv_row[:, bass.ts(tq, 128)],
                                wp[0:1, 192:193])
        svT = P.tile([128, NT], F32, tag="svT")
        nc.gpsimd.tensor_copy(out=svT[:], in_=pesv[:, 0:NT])

        # ---------- binary mask tiles: edges[j,i] * (j != i) ----------
        # (allocated here; built inside the main loop so DVE's build stream
        # is not delayed at the start)
        masks = [P.tile([128, N], BF16, tag=f"mask{jt}", name=f"mask{jt}")
                 for jt in range(NT)]

        def emit_mask(jt):
            mj = masks[jt]
            nc.vector.tensor_copy(out=mj[:], in_=esbs[jt])
            nc.vector.tensor_mul(out=mj[:, bass.ts(jt, 128)],
                                 in0=mj[:, bass.ts(jt, 128)], in1=ive[:])

        # ---------- main pass: j-major scores^T ----------
        # Pair p (i = 2p, 2p+1): its relu tile is the STATIONARY operand of 4
        # tiny matmuls (one per j-tile bank), rhs = the two 0.8*w2 columns ->
        # scores^T[j, 2p:2p+2] lands directly in [j, i] layout (no ET
        # transposes, denominators fused into agg).
        ET = [P.tile([128, N], BF16, tag=f"ET{jt}", name=f"ET{jt}") for jt in range(NT)]
        SCb = [SC.tile([128, N], F32, tag=f"sc{jt}", name=f"sc{jt}")
               for jt in range(NT)]
        pa_all = PA.tile([128, NT, H + 1], F32, tag="pa_all")

        def emit_pair(p):
            eng = PAT[p % len(PAT)]
            rl = RLOF[eng].tile([128, N], BF16, tag="relu")
            if eng == "v":
                nc.vector.tensor_scalar(out=rl[:], in0=Vrep[:],
                                        scalar1=U2[:, p:p + 1], scalar2=0.0,
                                        op0=OP.add, op1=OP.max)
            elif eng == "a":
                nc.scalar.activation(out=rl[:], in_=Vrep[:], func=AF.Relu,
                                     bias=U2[:, p:p + 1], scale=1.0)
            else:
                nc.gpsimd.tensor_scalar(out=rl[:], in0=Vrep[:],
                                        scalar1=U2[:, p:p + 1], scalar2=0.0,
                                        op0=OP.add, op1=OP.max)
            # disjoint 2-column slices: every matmul is its own psum group, so
            # the bank is never mid-group and exp can read finished columns
            for jt in range(NT):
                nc.tensor.matmul(SCb[jt][:, 2 * p:2 * p + 2],
                                 rl[:, bass.ts(jt, 128)], w2pair,
                                 start=True, stop=True)

        def emit_post(its):
            lo, hi = 128 * its[0], 128 * (its[-1] + 1)
            for jt in range(NT):
                Xc = XE.tile([128, hi - lo], BF16, tag="X", name="Xc")
                nc.scalar.activation(out=Xc[:], in_=SCb[jt][:, lo:hi], func=AF.Exp,
                                     bias=svT[:, jt:jt + 1], scale=1.0)
                nc.vector.tensor_mul(out=ET[jt][:, lo:hi], in0=Xc[:],
                                     in1=masks[jt][:, lo:hi])
            for it in its:
                for jt in range(NT):
                    nc.tensor.matmul(pa_all[:, it, :], ET[jt][:, bass.ts(it, 128)],
                                     nbe_aug[jt][:], start=(jt == 0),
                                     stop=(jt == NT - 1))
                den = pa_all[:, it, H:H + 1]
                gate = SM.tile([128, 1], F32, tag="gate", name="gate")
                nc.vector.tensor_single_scalar(out=gate[:], in_=den,
                                               scalar=1e-6, op=OP.is_gt)
                dsafe = SM.tile([128, 1], F32, tag="dsafe", name="dsafe")
                nc.vector.tensor_scalar_max(out=dsafe[:], in0=den, scalar1=1e-30)
                recipg = SM.tile([128, 1], F32, tag="recipg", name="recipg")
                nc.vector.reciprocal(out=recipg[:], in_=dsafe[:])
                sg = SM.tile([128, H], F32, tag="sg")
                nc.gpsimd.tensor_scalar_mul(out=sg[:], in0=selfe[it][:],
                                            scalar1=gate[:])
                nc.vector.tensor_mul(out=recipg[:], in0=recipg[:], in1=gate[:])
                ot = SM.tile([128, H], F32, tag="ot")
                nc.vector.scalar_tensor_tensor(out=ot[:], in0=pa_all[:, it, 0:H],
                                               scalar=recipg[:], in1=sg[:],
                                               op0=OP.mult, op1=OP.add)
                nc.sync.dma_start(out=t["out"].ap()[bass.ts(it, 128), :], in_=ot[:])

        for p in range(NPAIR):
            emit_pair(p)
            if p == 34:
                emit_mask(0)
                emit_mask(1)
            elif p == 66:
                emit_mask(2)
                emit_mask(3)
            elif p == 150:
                emit_post([0, 1])
            elif p == 214:
                emit_post([2])
        emit_post([3])


def _host_constants(inputs):
    f32 = np.float32
    bf = ml_dtypes.bfloat16
    H_ = H
    w2 = np.asarray(inputs["comb_w2"], f32)[:, 0]      # [H]

    wpack = np.zeros((128, 320), f32)
    wpack[:, 0:64] = np.asarray(inputs["self_w1"], f32)
    wpack[0:64, 64:128] = np.asarray(inputs["self_w2"], f32)
    wpack[0:64, 128:192] = np.asarray(inputs["comb_w1"], f32)[:H_]
    wpack[:, 192:320] = np.eye(128, dtype=f32)
    bvec = np.stack([
        np.asarray(inputs["self_b1"], f32),
        np.asarray(inputs["nb_b1"], f32),
        np.asarray(inputs["self_b2"], f32),
        np.asarray(inputs["nb_b2"], f32),
        np.asarray(inputs["comb_b1"], f32),
    ], axis=1)

    bfpack = np.zeros((128, 640), f32)
    bfpack[:, 0:64] = np.asarray(inputs["nb_w1"], f32)
    bfpack[0:64, 64:128] = np.asarray(inputs["nb_w2"], f32)
    bfpack[0:64, 128:192] = np.asarray(inputs["comb_w1"], f32)[H_:]
    bfpack[0:64, 192] = w2
    bfpack[:, 193:321] = np.eye(128, dtype=f32)
    bfpack[:, 321:449] = 1.0 - np.eye(128, dtype=f32)
    bfpack[0:64, 449 + 62] = 0.8 * w2
    bfpack[64:128, 449 + 63] = 0.8 * w2

    return {
        "wpack": wpack,
        "bvec": bvec,
        "bfpack": bfpack.astype(bf),
    }


def _build_fast_path(nc):
    """Cache a single jitted shard_map executable so repeat kernel() calls
    skip jax re-tracing (same lowering run_bass_kernel_spmd uses under axon)."""
    import jax
    from jax.sharding import Mesh, PartitionSpec
    from jax.experimental.shard_map import shard_map

    bass2jax.install_neuronx_cc_hook()
    pname = nc.partition_id_tensor.name if nc.partition_id_tensor else None
    in_names, out_names, out_avals = [], [], []
    for alloc in nc.m.functions[0].allocations:
        if not isinstance(alloc, mybir.MemoryLocationSet):
            continue
        name = alloc.memorylocations[0].name
        if alloc.kind == "ExternalInput":
            if name != pname:
                in_names.append(name)
        elif alloc.kind == "ExternalOutput":
            out_names.append(name)
            out_avals.append(jax.core.ShapedArray(tuple(alloc.tensor_shape),
                                                  mybir.dt.np(alloc.dtype)))
    all_names = in_names + out_names + ([pname] if pname else [])

    def _body(*args):
        operands = list(args)
        if pname is not None:
            operands.append(bass2jax.partition_id_tensor())
        return tuple(bass2jax._bass_exec_p.bind(
            *operands, out_avals=tuple(out_avals), in_names=tuple(all_names),
            out_names=tuple(out_names), lowering_input_output_aliases=(),
            sim_require_finite=True, sim_require_nnan=True, nc=nc))

    devices = jax.devices()[:NCORES]
    mesh = Mesh(np.asarray(devices), ("core",))
    n_io = len(in_names) + len(out_names)
    sharded = jax.jit(
        shard_map(_body, mesh=mesh, in_specs=(PartitionSpec("core"),) * n_io,
                  out_specs=(PartitionSpec("core"),) * len(out_names),
                  check_rep=False),
        keep_unused=True,
    )
    return sharded, in_names, out_names, out_avals


def kernel(**inputs):
    zb = all(not np.any(np.asarray(inputs[k]))
             for k in ("self_b1", "self_b2", "nb_b1", "nb_b2", "comb_b1"))
    first = ("nc", zb) not in _CACHE
    if first:
        _CACHE.clear()
        _CACHE[("nc", zb)] = _build_module(zero_bias=zb)
    nc = _CACHE[("nc", zb)]

    consts = _host_constants(inputs)
    nodes = np.asarray(inputs["nodes"], np.float32).reshape(B, N, D)
    edges = (np.asarray(inputs["edges"]) != 0).astype(np.uint8)

    in_maps = []
    for c in range(NCORES):
        m = dict(consts)
        m["nodes"] = np.ascontiguousarray(nodes[c].T)   # pre-transposed [D, N]
        m["edges"] = edges[c]
        in_maps.append(m)

    if first:
        res = run_bass_kernel_spmd(nc, in_maps, core_ids=list(range(NCORES)))
        _CACHE["fast"] = _build_fast_path(nc)
        return np.stack([res.results[c]["out"] for c in range(NCORES)]).astype(np.float32)

    import jax
    sharded, in_names, out_names, out_avals = _CACHE["fast"]
    ckey = hash(tuple((k, v.tobytes()) for k, v in sorted(consts.items())))
    if _CACHE.get("ckey") != ckey:
        _CACHE["cdev"] = {
            n: jax.device_put(np.concatenate([np.asarray(in_maps[c][n])
                                              for c in range(NCORES)], axis=0))
            for n in in_names if n not in ("nodes", "edges")
        }
        _CACHE["zdev"] = [jax.device_put(np.zeros((NCORES * a.shape[0], *a.shape[1:]),
                                                  a.dtype)) for a in out_avals]
        _CACHE["ckey"] = ckey
    cdev = _CACHE["cdev"]
    concat_in = [cdev[n] if n in cdev else
                 np.concatenate([np.asarray(in_maps[c][n]) for c in range(NCORES)], axis=0)
                 for n in in_names]
    outs = sharded(*concat_in, *_CACHE["zdev"])
    i = out_names.index("out")
    return np.asarray(outs[i]).reshape(NCORES, N, H).astype(np.float32)
